# revision 13
# baseline (speedup 1.0000x reference)
"""Trainium2 Bass kernel for a 12-layer prefix-causal transformer.

Sharding: data-parallel over batch B=8 across 8 NeuronCores (1 sequence per
core, weights replicated, no collectives).

Per-core strategy:
  - Residual x kept fp32 in SBUF as [128(t%128), 8(t-tile), 1024(d)].
  - LN scale/bias folded into the following GEMM's weights/bias on the host,
    so on-device LN is just (x-mean)*rstd (one scalar-engine op per tile).
  - QKV computed from h^T (PE-transposed LN output). Q^T / K^T are produced
    directly in [d_head(P), t] layout, so matmul(lhsT=K^T_h, rhs=Q^T_h)
    gives scores s^T in [kt(P), qt(free)] without transposing activations.
  - Softmax: exp on scalar engine (1/sqrt(hd) pre-folded into Q); a column of
    ones interleaved into V makes the AV matmul also produce Z = sum exp;
    normalization multiplies o'^T columns by 1/Z broadcast across partitions
    via a k=1 matmul.
  - Prefix-causal mask: sensor queries attend only sensor keys (kt tiles 0-3,
    unmasked); traj queries get causal handling via ragged tile ranges plus a
    constant 128x128 triangular mask on diagonal blocks.
  - FFN: transposed-mode GEMM1 gives u^T in [ff(P), t]; gelu(+bias) on the
    scalar engine; GEMM2 accumulates back into [t(P), d].
All matmuls are fp16 x fp16 with fp32 PSUM accumulation.

SBUF/PSUM pool reservations are static (bufs x max tile size per tag), so
big tensors share one 4-slot pool (qT, kT, v, oT, then the 4 gT quarters
reuse those slots) and PSUM uses exactly 8 banks: mm(4) + acc(3) + bc(1).
"""

import os
import sys
from contextlib import ExitStack

for _p in ("/opt/trn_rl_repo", "/root/.axon_site/_ro/trn_rl_repo"):
    if os.path.isdir(_p) and _p not in sys.path:
        sys.path.insert(0, _p)

import numpy as np

import concourse.bass as bass
import concourse.tile as tile
from concourse import bacc, mybir
from concourse.bass_utils import run_bass_kernel_spmd

dt = mybir.dt
AF = mybir.ActivationFunctionType
OP = mybir.AluOpType

B, LS = 8, 512
L, D, H, HD, FF = 1024, 1024, 16, 64, 4096
NL = int(os.environ.get("BASS_NL", "12"))
NT = L // 128    # 8 token tiles
ND = D // 128    # 8 d_model tiles
NF = FF // 128   # 32 ffn tiles
EPS = 1e-5

f32, f16 = dt.float32, dt.float16


def _ln_normalize(nc, small, out_ap, x_ap, epst):
    """out = (x - mean(x)) * rsqrt(var(x) + eps), per partition over 1024."""
    stats = small.tile([128, 12], f32, tag="stats")
    nc.vector.bn_stats(stats[:, 0:6], x_ap[:, 0:512])
    nc.vector.bn_stats(stats[:, 6:12], x_ap[:, 512:1024])
    mv = small.tile([128, 2], f32, tag="mv")
    nc.vector.bn_aggr(mv[:], stats[:])
    std = small.tile([128, 1], f32, tag="std")
    nc.scalar.activation(std[:], mv[:, 1:2], AF.Sqrt, bias=epst[:])
    rstd = small.tile([128, 1], f32, tag="rstd")
    nc.vector.reciprocal_approx_fast(rstd[:], std[:])
    nmr = small.tile([128, 1], f32, tag="nmr")
    # nmr = (mean * rstd) * -1
    nc.vector.tensor_scalar(nmr[:], mv[:, 0:1], rstd[:], -1.0, OP.mult, OP.mult)
    # (x * rstd) + nmr on DVE (keeps the scalar engine free for exp/gelu)
    nc.vector.tensor_scalar(out_ap, x_ap, rstd[:], nmr[:], OP.mult, OP.add)


def build_nc(nl=NL, has_qb=True, has_ob=True, has_b1=True, has_b2=True):
    nc = bacc.Bacc("TRN2", target_bir_lowering=False, debug=False, num_devices=8)

    x_d = nc.dram_tensor("x0", [L, D], f32, kind="ExternalInput")
    pos_d = nc.dram_tensor("pos", [L, D], f32, kind="ExternalInput")
    wqk_d = nc.dram_tensor("wqk", [nl, 16, 128, ND, 128], f16, kind="ExternalInput")
    wqv_d = nc.dram_tensor("wqv", [nl, 128, ND, 1024], f16, kind="ExternalInput")
    bq_d = nc.dram_tensor("bq", [nl, 128, ND], f32, kind="ExternalInput")
    wo_d = nc.dram_tensor("wo", [nl, D, D], f16, kind="ExternalInput")
    bo_d = nc.dram_tensor("bo", [nl, 128, D], f16, kind="ExternalInput")
    w1_d = nc.dram_tensor("w1", [nl, NF, 128, ND, 128], f16, kind="ExternalInput")
    b1_d = nc.dram_tensor("b1", [nl, 128, NF], f32, kind="ExternalInput")
    w2_d = nc.dram_tensor("w2", [nl, 2, NF, 128, 512], f16, kind="ExternalInput")
    b2_d = nc.dram_tensor("b2", [nl, 128, D], f16, kind="ExternalInput")
    fs_d = nc.dram_tensor("flns", [128, D], f32, kind="ExternalInput")
    fb_d = nc.dram_tensor("flnb", [128, D], f32, kind="ExternalInput")
    id_d = nc.dram_tensor("ident", [128, 128], f16, kind="ExternalInput")
    mk_d = nc.dram_tensor("mask", [128, 128], f16, kind="ExternalInput")
    out_d = nc.dram_tensor("out", [L, D], f32, kind="ExternalOutput")

    with tile.TileContext(nc) as tc, ExitStack() as ctx:
        def pool(name, bufs, space="SBUF"):
            return ctx.enter_context(tc.tile_pool(name=name, bufs=bufs, space=space))

        cpool = pool("consts", 1)       # ident, mask, ones64, epst
        small = pool("small", 16)       # LN stats etc, <=48B tags
        rzp = pool("rz", 2)             # [1,512] f16
        bcp = pool("bcs", 2)            # [64,512] f32
        xpool = pool("x", 1)            # residual fp32, 32KB
        f4k = pool("f4k", 2)            # pos-load + final-LN consts, f32 4KB
        hpool = pool("h", 2)            # LN output per t-tile, f16 2KB
        htpool = pool("hT", 1)          # transposed LN output, 16KB
        big = pool("big", 4)            # qT,kT,v,oT + 4 gT quarters, 16.25KB
        apool = pool("aT", 3)           # exp(scores) per (head,chunk), 8KB
        wsp = pool("wstream", 3)        # streamed QK/W1 weight tiles, 2KB
        w16 = pool("w16", 1)            # V-part weights / out-proj weights 16KB
        w2p = pool("w2s", 4)            # streamed GEMM2 weight tiles, 1KB
        bpool = pool("bias", 1)         # per-layer bias tiles
        psum = pool("ps", 4, space="PSUM")    # tag mm: 4 banks
        psac = pool("psac", 4, space="PSUM")  # tag acc: AV + GEMM2: 4 banks
        dpool = pool("dscr", 4, space="DRAM")  # 1/Z rows bounced via DRAM

        ident = cpool.tile([128, 128], f16)
        nc.sync.dma_start(ident[:], id_d[:])
        mask = cpool.tile([128, 128], f16)
        nc.sync.dma_start(mask[:], mk_d[:])
        epst = cpool.tile([128, 1], f32)
        nc.gpsimd.memset(epst[:], EPS)

        # ---- initial x = concat(sensor, traj) + pos ----
        x = xpool.tile([128, NT, D], f32)
        for i in range(NT):
            nc.sync.dma_start(x[:, i, :], x_d[i * 128:(i + 1) * 128, :])
            pt = f4k.tile([128, D], f32, tag="f4k")
            nc.sync.dma_start(pt[:], pos_d[i * 128:(i + 1) * 128, :])
            nc.vector.tensor_add(x[:, i, :], x[:, i, :], pt[:])

        def ln_transpose(src_x, hT):
            """LN each t-tile of src_x, PE-transpose into hT [128, ND, L] f16."""
            for i in range(NT):
                hi = hpool.tile([128, D], f16, tag="h")
                _ln_normalize(nc, small, hi[:], src_x[:, i, :], epst)
                tp = psum.tile([128, ND, 128], f16, tag="mm")
                for j in range(ND):
                    nc.tensor.transpose(
                        tp[:, j, :], hi[:, j * 128:(j + 1) * 128], ident[:]
                    )
                nc.vector.tensor_copy(hT[:, :, i * 128:(i + 1) * 128], tp[:])

        for l in range(nl):
            # ================= attention =================
            hT = htpool.tile([128, ND, L], f16, tag="hT")
            ln_transpose(x, hT)

            if has_qb:
                bq_sb = bpool.tile([128, ND], f32, tag="bq")
                nc.sync.dma_start(bq_sb[:], bq_d[l])

            # V in normal layout [t(P), o], interleaved 64+1 (ones col per head)
            wqv = w16.tile([128, ND, 1024], f16, tag="w16")
            nc.sync.dma_start(wqv[:], wqv_d[l])
            v = big.tile([128, NT, 16 * 65], f16, tag="big")
            for i in range(NT):
                vv = v[:, i, :].rearrange("p (h e) -> p h e", e=65)
                nc.gpsimd.memset(vv[:, :, 64:65], 1.0)
                for c in range(2):
                    pt = psum.tile([128, 512], f32, tag="mm")
                    for j in range(ND):
                        nc.tensor.matmul(
                            pt[:],
                            hT[:, j, i * 128:(i + 1) * 128],
                            wqv[:, j, c * 512:(c + 1) * 512],
                            start=(j == 0),
                            stop=(j == ND - 1),
                        )
                    nc.vector.tensor_copy(
                        vv[:, 8 * c:8 * (c + 1), 0:64],
                        pt[:].rearrange("p (h e) -> p h e", e=64),
                    )

            # wo prefetch (slot frees once wqv is released after V GEMM)
            wo_sb = w16.tile([128, ND, D], f16, tag="w16")
            for j in range(ND):
                nc.sync.dma_start(wo_sb[:, j, :], wo_d[l, j * 128:(j + 1) * 128, :])
            if has_ob:
                bo_sb = bpool.tile([128, D], f16, tag="bo")
                nc.sync.dma_start(bo_sb[:], bo_d[l])

            # QKV (Q^T/K^T transposed layout, streamed weights) interleaved
            # with per-head attention: after Q/K o-tile mi is evacuated, heads
            # 2mi and 2mi+1 are emitted so their exp (scalar engine) overlaps
            # the next o-tile's GEMMs on the PE.
            qT = big.tile([128, ND, 1040], f16, tag="big")
            kT = big.tile([128, ND, 1040], f16, tag="big")
            oT = big.tile([128, ND, 1040], f16, tag="big")

            def qk_tile(m):
                wt = wsp.tile([128, ND, 128], f16, tag="ws", name=f"wt_{l}_{m}")
                nc.sync.dma_start(wt[:], wqk_d[l, m])
                dst = qT if m < ND else kT
                for c in range(2):
                    pt = psum.tile([128, 512], f32, tag="mm", name=f"pt_{l}_{m}_{c}")
                    for j in range(ND):
                        nc.tensor.matmul(
                            pt[:],
                            wt[:, j, :],
                            hT[:, j, c * 512:(c + 1) * 512],
                            start=(j == 0),
                            stop=(j == ND - 1),
                        )
                    if m < ND and has_qb:  # Q bias (scale folded into weights)
                        nc.scalar.activation(
                            dst[:, m, c * 512:(c + 1) * 512], pt[:],
                            AF.Identity, bias=bq_sb[:, m:m + 1],
                        )
                    else:
                        nc.vector.tensor_copy(
                            dst[:, m % ND, c * 512:(c + 1) * 512], pt[:]
                        )

            def head(hh):
                jo, po = hh // 2, 64 * (hh % 2)
                kTh = kT[po:po + 64, jo, :]
                qTh = qT[po:po + 64, jo, :]

                aTs = []
                for c in range(2):
                    nkt = 4 if c == 0 else 8
                    aT = apool.tile([128, 8, 512], f16, tag="aT",
                                    name=f"aT_{l}_{hh}_{c}")
                    for j in range(nkt):
                        w0 = max(0, (j - 4) * 128)
                        sp = psum.tile([128, 512], f32, tag="mm")
                        nc.tensor.matmul(
                            sp[:, w0:512],
                            kTh[:, j * 128:(j + 1) * 128],
                            qTh[:, c * 512 + w0:(c + 1) * 512],
                            start=True, stop=True,
                        )
                        nc.scalar.activation(aT[:, j, w0:512], sp[:, w0:512], AF.Exp)
                        if c == 1 and j >= 4:
                            nc.vector.tensor_mul(
                                aT[:, j, w0:w0 + 128], aT[:, j, w0:w0 + 128], mask[:]
                            )
                    aTs.append(aT)
                for c in range(2):
                    nkt = 4 if c == 0 else 8
                    aT = aTs[c]
                    # AV (+ Z row): o'^T[65, qt] accumulated over kt tiles
                    op = psac.tile([128, 512], f32, tag="acc")
                    for j in range(nkt):
                        w0 = max(0, (j - 4) * 128)
                        nc.tensor.matmul(
                            op[0:65, w0:512],
                            v[:, j, 65 * hh:65 * hh + 65],
                            aT[:, j, w0:512],
                            start=(j == 0),
                            stop=(j == nkt - 1),
                        )
                    # 1/Z on DVE (keeps ACT in the exp table set; Ln would
                    # force a ~2.7us table reload per use); broadcast across
                    # partitions by bouncing the row through DRAM (off the PE)
                    rz = rzp.tile([1, 512], f32, tag="rz")
                    nc.vector.reciprocal(rz[:], op[64:65, :])
                    rzd = dpool.tile([512], f32, tag="rzd",
                                     name=f"rzd_{l}_{hh}_{c}")
                    nc.sync.dma_start(rzd[:].unsqueeze(0), rz[:])
                    bcs = bcp.tile([64, 512], f32, tag="bcs")
                    nc.sync.dma_start(
                        bcs[:], rzd[:].unsqueeze(0).broadcast_to((64, 512))
                    )
                    nc.vector.tensor_mul(
                        oT[po:po + 64, jo, c * 512:(c + 1) * 512], op[0:64, :], bcs[:]
                    )

            for mi in range(ND):
                qk_tile(mi)
                qk_tile(ND + mi)
                head(2 * mi)
                head(2 * mi + 1)

            # out-proj + residual (+ folded V-bias/out-bias)
            for i in range(NT):
                for c in range(2):
                    cs = slice(c * 512, (c + 1) * 512)
                    yp = psum.tile([128, 512], f32, tag="mm")
                    for j in range(ND):
                        nc.tensor.matmul(
                            yp[:],
                            oT[:, j, i * 128:(i + 1) * 128],
                            wo_sb[:, j, cs],
                            start=(j == 0),
                            stop=(j == ND - 1),
                        )
                    nc.vector.tensor_add(x[:, i, cs], x[:, i, cs], yp[:])
                    if has_ob:
                        nc.vector.tensor_add(x[:, i, cs], x[:, i, cs], bo_sb[:, cs])

            # ================= FFN =================
            h2T = htpool.tile([128, ND, L], f16, tag="hT")
            ln_transpose(x, h2T)

            if has_b1:
                b1_sb = bpool.tile([128, NF], f32, tag="b1")
                nc.sync.dma_start(b1_sb[:], b1_d[l])
            if has_b2:
                b2_sb = bpool.tile([128, D], f16, tag="b2")
                nc.sync.dma_start(b2_sb[:], b2_d[l])

            # GEMM1 (transposed out) + gelu -> g^T [ff(P), t] in 4 quarters
            gq = []
            for q in range(4):
                g = big.tile([128, 8, 1040], f16, tag="big", name=f"gT_{l}_{q}")
                gq.append(g)
            for f in range(NF):
                w1t = wsp.tile([128, ND, 128], f16, tag="ws")
                nc.sync.dma_start(w1t[:], w1_d[l, f])
                for c in range(2):
                    gp = psum.tile([128, 512], f32, tag="mm")
                    for j in range(ND):
                        nc.tensor.matmul(
                            gp[:],
                            w1t[:, j, :],
                            h2T[:, j, c * 512:(c + 1) * 512],
                            start=(j == 0),
                            stop=(j == ND - 1),
                        )
                    if has_b1:
                        nc.scalar.activation(
                            gq[f // 8][:, f % 8, c * 512:(c + 1) * 512], gp[:],
                            AF.Gelu, bias=b1_sb[:, f:f + 1],
                        )
                    else:
                        nc.scalar.activation(
                            gq[f // 8][:, f % 8, c * 512:(c + 1) * 512], gp[:], AF.Gelu,
                        )

            # GEMM2: accumulate over all 32 ff tiles; t-groups of 4 (4 banks),
            # c inner so x t-tiles complete early for the next layer's LN
            for tg in ((0, 1, 2, 3), (4, 5, 6, 7)):
                for c in range(2):
                    cs = slice(c * 512, (c + 1) * 512)
                    ys = [psac.tile([128, 512], f32, tag="acc",
                                    name=f"psy_{l}_{c}_{tg[0]}_{k}")
                          for k in range(len(tg))]
                    for f in range(NF):
                        w2t = w2p.tile([128, 512], f16, tag="w2s")
                        nc.sync.dma_start(w2t[:], w2_d[l, c, f])
                        for k, i in enumerate(tg):
                            nc.tensor.matmul(
                                ys[k][:],
                                gq[f // 8][:, f % 8, i * 128:(i + 1) * 128],
                                w2t[:],
                                start=(f == 0),
                                stop=(f == NF - 1),
                            )
                    for k, i in enumerate(tg):
                        nc.vector.tensor_add(x[:, i, cs], x[:, i, cs], ys[k][:])
                        if has_b2:
                            nc.vector.tensor_add(x[:, i, cs], x[:, i, cs], b2_sb[:, cs])

        # ================= final LN (in-place on x) + output =================
        flns = f4k.tile([128, D], f32, tag="f4k")
        nc.sync.dma_start(flns[:], fs_d[:])
        flnb = f4k.tile([128, D], f32, tag="f4k")
        nc.sync.dma_start(flnb[:], fb_d[:])
        for i in range(NT):
            _ln_normalize(nc, small, x[:, i, :], x[:, i, :], epst)
            nc.vector.tensor_mul(x[:, i, :], x[:, i, :], flns[:])
            nc.vector.tensor_add(x[:, i, :], x[:, i, :], flnb[:])
            nc.sync.dma_start(out_d[i * 128:(i + 1) * 128, :], x[:, i, :])

    nc.compile()
    return nc


def _host_prep(sensor_tokens, traj_tokens, pos_embed, ln1_s, ln1_b,
               qkv_w, qkv_b, out_w, out_b, ln2_s, ln2_b,
               w1, b1, w2, b2, fln_s, fln_b, nl=NL):
    """Fold LN affine params into weights; transpose/retile + fp16-cast."""
    fp = np.float32
    x_all = np.concatenate([sensor_tokens, traj_tokens], axis=1).astype(fp)  # [B,L,D]
    pos = np.ascontiguousarray(pos_embed[:L]).astype(fp)

    wqk = np.empty((nl, 16, 128, ND, 128), np.float16)
    wqv = np.empty((nl, 128, ND, 1024), np.float16)
    bqh = np.empty((nl, 128, ND), fp)
    woT = np.empty((nl, D, D), np.float16)
    boh = np.empty((nl, 128, D), np.float16)
    w1T = np.empty((nl, NF, 128, ND, 128), np.float16)
    b1h = np.empty((nl, 128, NF), fp)
    w2T = np.empty((nl, 2, NF, 128, 512), np.float16)
    b2h = np.empty((nl, 128, D), np.float16)

    for i in range(nl):
        Wq = qkv_w[i].astype(fp)                                  # [3D, D]
        bfull = qkv_b[i].astype(fp) + Wq @ ln1_b[i].astype(fp)    # [3D]
        Wq = Wq * ln1_s[i].astype(fp)[None, :]
        WqT = Wq.T.astype(np.float16)                             # [D, 3D]
        # Q/K part: per 128-col m-tile -> [m, 128(p=d%128), ND(j=d//128), 128]
        WqT[:, :D] *= np.float16(0.125)  # fold attention scale into Q weights
        qk = WqT[:, :2 * D].reshape(ND, 128, 16, 128)             # j,p,m,col
        wqk[i] = qk.transpose(2, 1, 0, 3)
        wqv[i] = WqT[:, 2 * D:].reshape(ND, 128, 1024).transpose(1, 0, 2)
        bq = bfull[:D] * 0.125
        bqh[i] = bq.reshape(ND, 128).T
        bv = bfull[2 * D:]
        Wo = out_w[i].astype(fp)                                  # [D, D]
        bo = out_b[i].astype(fp) + Wo @ bv
        woT[i] = Wo.T.astype(np.float16)
        boh[i] = np.broadcast_to(bo.astype(np.float16), (128, D))
        W1 = w1[i].astype(fp)                                     # [FF, D]
        b1f = b1[i].astype(fp) + W1 @ ln2_b[i].astype(fp)
        W1 = W1 * ln2_s[i].astype(fp)[None, :]
        W1T = W1.T.astype(np.float16)                             # [D, FF]
        w1T[i] = W1T.reshape(ND, 128, NF, 128).transpose(2, 1, 0, 3)
        b1h[i] = b1f.reshape(NF, 128).T
        W2T = w2[i].astype(fp).T.astype(np.float16)               # [FF, D]
        w2T[i] = W2T.reshape(NF, 128, 2, 512).transpose(2, 0, 1, 3)
        b2h[i] = np.broadcast_to(b2[i].astype(np.float16), (128, D))

    common = dict(
        pos=pos,
        wqk=wqk, wqv=wqv, bq=bqh, wo=woT, bo=boh,
        w1=w1T, b1=b1h, w2=w2T, b2=b2h,
        flns=np.broadcast_to(fln_s.astype(fp), (128, D)).copy(),
        flnb=np.broadcast_to(fln_b.astype(fp), (128, D)).copy(),
        ident=np.eye(128, dtype=np.float16),
        mask=np.triu(np.ones((128, 128), np.float16)),
    )
    in_maps = [dict(common, x0=np.ascontiguousarray(x_all[c])) for c in range(B)]
    return in_maps


_NC = {}
LAST_RESULT = None


def kernel(**inputs):
    global LAST_RESULT
    in_maps = _host_prep(**inputs)
    m0 = in_maps[0]
    flags = (bool(np.any(m0["bq"])), bool(np.any(m0["bo"])),
             bool(np.any(m0["b1"])), bool(np.any(m0["b2"])))
    if flags not in _NC:
        _NC[flags] = build_nc(NL, *flags)
    res = run_bass_kernel_spmd(_NC[flags], in_maps, core_ids=list(range(B)))
    LAST_RESULT = res
    return np.stack([res.results[c]["out"] for c in range(B)]).astype(np.float32)
